# revision 13
# baseline (speedup 1.0000x reference)
"""ChebConv (K=3) kernel for Trainium2, data-parallel over batch across 8 NeuronCores.

Math (per batch b):
    d    = adj.sum(axis=1)  (row sums);  s = d^-0.5;  q = s^2
    M    = Diag(s) A Diag(s);  L = I - M
    Tx0 = x, Tx1 = x - Mx, Tx2 = 2(Tx1 - M Tx1) - x
    out  = relu(sum_k Txk @ W[k] + bsum)
  (the reference's +1e-6 under the sqrt is ~1e-9 relative for this input
   distribution - far below bf16 noise - so it is folded away here.)

Kernel-side dataflow (everything bf16 except PSUM accum; yk := Diag(s) Txk):
    ats2[j,i]  = A[i,j] q[i]              (PE transpose per row-tile, rhs=diag(q))
    y0n[j,f]   = s[j] x[j,f]
    w1T[f,i]   = sum_j y0n[j,f] ats2[j,i]  = s[i](Mx)[i,f]          (pass 1)
    y1T = y0T - w1T;  y1n = y0n - nat(w1T)      (plain subtracts - q is folded)
    w2T[f,i]   = sum_j y1n[j,f] ats2[j,i]  = s[i](M Tx1)[i,f]       (pass 2)
    y2T = (2 y1T - y0T) - 2 w2T
    outT[fo,i] = sum_k sum_f W[k][f,fo] ykT[f,i]   (lhsT = W natural, rhs = ykT)
    out[i,fo]  = relu(dsq[i] * nat(outT) + bsum)   (dsq = d^0.5)

Performance structure:
  - adj streams via gpsimd SWDGE with fp32->bf16 cast during DMA (8 x 2MB chunks);
    x/consts ride the scalar HWDGE ring (bf16 W/ident prepared host-side);
    out stores ride sync.
  - the per-tile rowsum is split across engines: DVE tensor_reduce on one half,
    ACT activation+accum_out on the other, then dsq = Sqrt(d_a*1 + d_b) via the
    activation bias AP - halves the per-engine cost and latency of the chain
    that gates the diag(q) transposes.
  - pass-1 runs triangularly inside the stream loop; pass-2 per 512-column
    group with a pipelined epilogue (copy, y2T, output matmuls, transpose-back,
    scale+relu, store).
  - PSUM->SBUF copies are [128,1024] ops alternating ACT/DVE (ACT-biased 5:3).
"""

import numpy as np

B, N, F, K = 8, 2048, 128, 3
P = 128
NT = N // P   # 16 node tiles
NCH = NT // 2  # 8 dma chunks of 2 tiles
NCORES = 8

_cache = {}


def _build_nc():
    from contextlib import ExitStack

    import concourse.bacc as bacc
    import concourse.tile as tile
    from concourse import mybir
    from concourse.tile import add_dep_helper

    f32 = mybir.dt.float32
    bf16 = mybir.dt.bfloat16
    AF = mybir.ActivationFunctionType
    OP = mybir.AluOpType
    AX = mybir.AxisListType

    nc = bacc.Bacc("TRN2", target_bir_lowering=False, debug=False, num_devices=NCORES)
    adj = nc.dram_tensor("adj", [N, N], f32, kind="ExternalInput").ap()
    x = nc.dram_tensor("x", [N, F], f32, kind="ExternalInput").ap()
    W = nc.dram_tensor("W", [P, K, F], bf16, kind="ExternalInput").ap()
    bsum_d = nc.dram_tensor("bsum", [P, F], f32, kind="ExternalInput").ap()
    ident = nc.dram_tensor("ident", [P, P], bf16, kind="ExternalInput").ap()
    out = nc.dram_tensor("out", [N, F], f32, kind="ExternalOutput").ap()
    out_t = out.rearrange("(t p) f -> p t f", p=P)
    adj_r = adj.rearrange("(r p) n -> p r n", p=P)
    x_t = x.rearrange("(t p) f -> p t f", p=P)

    with ExitStack() as ctx:
        tc = ctx.enter_context(tile.TileContext(nc))
        consts = ctx.enter_context(tc.tile_pool(name="consts", bufs=1))
        ap_ = ctx.enter_context(tc.tile_pool(name="achunk", bufs=5))
        big = ctx.enter_context(tc.tile_pool(name="big", bufs=1))
        small = ctx.enter_context(tc.tile_pool(name="small", bufs=4))
        scr = ctx.enter_context(tc.tile_pool(name="scr", bufs=2))
        ps_acc = ctx.enter_context(tc.tile_pool(name="ps_acc", bufs=1, space="PSUM"))
        ps_t = ctx.enter_context(tc.tile_pool(name="ps_t", bufs=2, space="PSUM"))

        # ---- constants -------------------------------------------------
        x_f = consts.tile([P, NT, F], f32)
        nc.scalar.dma_start(out=x_f, in_=x_t)
        ident_bf = consts.tile([P, P], bf16)
        nc.scalar.dma_start(out=ident_bf, in_=ident)
        w_bf = consts.tile([P, K, F], bf16)
        nc.scalar.dma_start(out=w_bf, in_=W)
        bsum = consts.tile([P, F], f32)
        nc.scalar.dma_start(out=bsum, in_=bsum_d)

        # per-node scalars, [P, NT]: column r holds values for node tile r
        dsq = consts.tile([P, NT], f32)
        sinv = consts.tile([P, NT], f32)

        y0n = big.tile([P, NT, F], bf16)
        y1n = big.tile([P, NT, F], bf16)
        ats2 = big.tile([P, NT, N], bf16)  # [j_in_tile, c(j tile), i]: A[i,j]q[i]
        y0T = big.tile([P, N], bf16)
        y1T = big.tile([P, N], bf16)
        ttT = big.tile([P, N], bf16)
        y2T = big.tile([P, N], bf16)
        w1bf = big.tile([P, N], bf16)
        w2bf = big.tile([P, N], bf16)
        oTbf = big.tile([P, N], bf16)

        z1 = ps_acc.tile([P, N], f32, tag="acc")

        cp = {"i": 0}

        def alt_copy(out, in_):
            # PSUM->SBUF copies: ACT-biased alternation (ACT is cheaper per
            # copy and DVE carries the other per-tile elementwise work)
            cp["i"] += 1
            if cp["i"] % 8 in (0, 2, 4, 5, 6):
                nc.scalar.copy(out=out, in_=in_)
            else:
                nc.vector.tensor_copy(out=out, in_=in_)

        # dummy sqrt so the ACT table load happens in the preamble
        warm = small.tile([P, 1], f32, tag="da")
        nc.vector.memset(warm, 1.0)
        nc.scalar.activation(out=warm, in_=warm, func=AF.Sqrt)

        prev_diag2 = None
        prev_sqrt = None

        # ---- streaming phase -------------------------------------------
        # chunk plan: tiles 0 and 1 load alone (compute starts sooner),
        # then 2-tile (2MB) chunks for DMA efficiency
        plan = [(0, 1), (1, 1)] + [(t, 2) for t in range(2, NT, 2)]
        for base, ntile in plan:
            a_t = ap_.tile([P, ntile, N], bf16, tag="a")
            nc.gpsimd.dma_start(out=a_t, in_=adj_r[:, base:base + ntile, :])
            for t2 in range(ntile):
                r = base + t2
                at_r = a_t[:, t2, :]
                # split rowsum: ACT does [0,1024) via accum, DVE [1024,2048)
                d_b = small.tile([P, 1], f32, tag="db")
                junk = scr.tile([P, N // 2], bf16, tag="junk")
                i_junk = nc.scalar.activation(out=junk, in_=at_r[:, 0:N // 2],
                                              func=AF.Identity, accum_out=d_b)
                d_a = small.tile([P, 1], f32, tag="da")
                i_red = nc.vector.tensor_reduce(out=d_a, in_=at_r[:, N // 2:N],
                                                axis=AX.X, op=OP.add)
                # dsq = sqrt(d_a + d_b)  (bias AP adds the other half)
                i_sqrt = nc.scalar.activation(out=dsq[:, r:r + 1], in_=d_a,
                                              func=AF.Sqrt, bias=d_b)
                nc.vector.reciprocal(out=sinv[:, r:r + 1], in_=dsq[:, r:r + 1])
                diag2 = small.tile([P, P], bf16, tag="diag")
                i_diag = nc.vector.tensor_scalar(out=diag2, in0=ident_bf,
                                                 scalar1=sinv[:, r:r + 1],
                                                 scalar2=sinv[:, r:r + 1],
                                                 op0=OP.mult, op1=OP.mult)
                # pin scheduler order so tile r's scalar chain is not queued
                # behind tile r+1's DMA-gated ops on the same engine
                if prev_diag2 is not None:
                    add_dep_helper(i_red.ins, prev_diag2.ins, sync=False,
                                   reason="keep DVE chain order across tiles")
                    add_dep_helper(i_junk.ins, prev_sqrt.ins, sync=False,
                                   reason="keep ACT chain order across tiles")
                prev_diag2 = i_diag
                prev_sqrt = i_sqrt
                # y0n = s * x, fp32 source, bf16 out (cast folded in)
                nc.vector.tensor_scalar(out=y0n[:, r, :], in0=x_f[:, r, :],
                                        scalar1=sinv[:, r:r + 1], scalar2=None,
                                        op0=OP.mult)
                # transpose + q[i]-scale A tile row r: 16 (128x128) matmuls
                for g in range(2):
                    pt = ps_t.tile([P, 8, P], f32, tag="t")
                    for qq in range(8):
                        c = 8 * g + qq
                        nc.tensor.matmul(pt[:, qq, :],
                                         lhsT=at_r[:, c * P:(c + 1) * P],
                                         rhs=diag2, start=True, stop=True)
                    alt_copy(ats2[:, 8 * g:8 * g + 8, r * P:(r + 1) * P], pt)

                # transpose y0n tiles into y0T once 4 are ready
                if r % 4 == 3:
                    pt_y0 = ps_t.tile([P, 8, P], f32, tag="t")
                    for qq in range(4):
                        nc.tensor.matmul(pt_y0[:, qq, :], lhsT=y0n[:, r - 3 + qq, :],
                                         rhs=ident_bf, start=True, stop=True)
                    alt_copy(y0T[:, (r - 3) * P:(r + 1) * P], pt_y0[:, 0:4, :])

                # triangular pass-1 terms that became ready with tile r:
                # (a) column block r, strips c <= r.
                # start=True clears has_written for the WHOLE bank (4 column
                # blocks), so only the bank's first-ever matmul may set it.
                for c in range(r + 1):
                    nc.tensor.matmul(z1[:, r * P:(r + 1) * P], lhsT=y0n[:, c, :],
                                     rhs=ats2[:, c, r * P:(r + 1) * P],
                                     start=(r % 4 == 0 and c == 0),
                                     stop=(r == NT - 1 and c == NT - 1),
                                     skip_group_check=True)
                # (b) new strip r into older column blocks (bank chunks)
                for sg in range((r + 3) // 4):
                    lo = 4 * sg
                    hi = min(lo + 4, r)  # blocks [lo, hi)
                    nc.tensor.matmul(z1[:, lo * P:hi * P], lhsT=y0n[:, r, :],
                                     rhs=ats2[:, r, lo * P:hi * P],
                                     start=False, stop=(r == NT - 1),
                                     skip_group_check=True)

        # ---- y1 from w1 = z1: y1T = y0T - w1T; y1n = y0n - nat(w1T) ----
        for g in range(4):
            alt_copy(w1bf[:, g * 512:(g + 1) * 512], z1[:, g * 512:(g + 1) * 512])
        nc.vector.tensor_tensor(out=y1T, in0=y0T, in1=w1bf, op=OP.subtract)
        for g in range(4):
            pt = ps_t.tile([P, 8, P], f32, tag="t")
            for qq in range(4):
                rr = 4 * g + qq
                nc.tensor.matmul(pt[:, qq, :], lhsT=w1bf[:, rr * P:(rr + 1) * P],
                                 rhs=ident_bf, start=True, stop=True)
            nc.vector.tensor_tensor(out=y1n[:, 4 * g:4 * g + 4, :],
                                    in0=y0n[:, 4 * g:4 * g + 4, :],
                                    in1=pt[:, 0:4, :], op=OP.subtract)
        nc.vector.scalar_tensor_tensor(out=ttT, in0=y1T, scalar=2.0, in1=y0T,
                                       op0=OP.mult, op1=OP.subtract)

        # ---- pass 2 by column group with pipelined epilogue ------------
        z2 = ps_acc.tile([P, N], f32, tag="acc")
        ykT = (y0T, y1T, y2T)
        for g in range(4):
            gl, gh = g * 512, (g + 1) * 512
            for c in range(NT):
                nc.tensor.matmul(z2[:, gl:gh], lhsT=y1n[:, c, :],
                                 rhs=ats2[:, c, gl:gh],
                                 start=(c == 0), stop=(c == NT - 1))
            alt_copy(w2bf[:, gl:gh], z2[:, gl:gh])
            nc.vector.scalar_tensor_tensor(out=y2T[:, gl:gh], in0=w2bf[:, gl:gh],
                                           scalar=-2.0, in1=ttT[:, gl:gh],
                                           op0=OP.mult, op1=OP.add)
            oT = ps_t.tile([P, 512], f32, tag="t")
            for k3 in range(K):
                nc.tensor.matmul(oT, lhsT=w_bf[:, k3, :], rhs=ykT[k3][:, gl:gh],
                                 start=(k3 == 0), stop=(k3 == K - 1))
            alt_copy(oTbf[:, gl:gh], oT)
            on = ps_t.tile([P, 8, P], f32, tag="t")
            for qq in range(4):
                rr = 4 * g + qq
                nc.tensor.matmul(on[:, qq, :], lhsT=oTbf[:, rr * P:(rr + 1) * P],
                                 rhs=ident_bf, start=True, stop=True)
            og = small.tile([P, 4, F], f32, tag="og")
            for qq in range(4):
                rr = 4 * g + qq
                tmp = small.tile([P, F], f32, tag="tmp")
                nc.vector.scalar_tensor_tensor(out=tmp, in0=on[:, qq, :],
                                               scalar=dsq[:, rr:rr + 1], in1=bsum,
                                               op0=OP.mult, op1=OP.add)
                nc.scalar.activation(out=og[:, qq, :], in_=tmp, func=AF.Relu)
            nc.sync.dma_start(out=out_t[:, 4 * g:4 * g + 4, :], in_=og)

    nc.compile()
    return nc


def _get_nc():
    if "nc" not in _cache:
        _cache["nc"] = _build_nc()
    return _cache["nc"]


def make_in_maps(x, adj, W, b):
    import ml_dtypes

    bf16 = ml_dtypes.bfloat16
    ident = np.ascontiguousarray(np.eye(P, dtype=np.float32).astype(bf16))
    x = np.ascontiguousarray(np.asarray(x, dtype=np.float32))
    adj = np.ascontiguousarray(np.asarray(adj, dtype=np.float32))
    # [K, in, out] -> [in, K, out], bf16 (the lhsT layout the kernel wants)
    Wf = np.ascontiguousarray(
        np.asarray(W, dtype=np.float32).transpose(1, 0, 2).astype(bf16))
    bf = np.asarray(b, dtype=np.float32)
    bsum = np.ascontiguousarray(
        np.broadcast_to(bf.sum(axis=0), (P, F)).astype(np.float32))
    return [
        {"adj": adj[c], "x": x[c], "W": Wf, "bsum": bsum, "ident": ident}
        for c in range(NCORES)
    ]


def run_raw(x, adj, W, b, **kwargs):
    from concourse import bass_utils

    nc = _get_nc()
    in_maps = make_in_maps(x, adj, W, b)
    res = bass_utils.run_bass_kernel_spmd(nc, in_maps,
                                          core_ids=list(range(NCORES)), **kwargs)
    out = np.stack([res.results[c]["out"] for c in range(NCORES)], axis=0)
    return out.astype(np.float32), res


def kernel(x, adj, W, b):
    out, _ = run_raw(x, adj, W, b)
    return out


# revision 15
# speedup vs baseline: 1.0004x; 1.0004x over previous
"""ChebConv (K=3) kernel for Trainium2, data-parallel over batch across 8 NeuronCores.

Math (per batch b):
    d    = adj.sum(axis=1)  (row sums);  s = d^-0.5;  q = s^2
    M    = Diag(s) A Diag(s);  L = I - M
    Tx0 = x, Tx1 = x - Mx, Tx2 = 2(Tx1 - M Tx1) - x
    out  = relu(sum_k Txk @ W[k] + bsum)
  (the reference's +1e-6 under the sqrt is ~1e-9 relative for this input
   distribution - far below bf16 noise - so it is folded away here.)

Kernel-side dataflow (everything bf16 except PSUM accum; yk := Diag(s) Txk):
    ats2[j,i]  = A[i,j] q[i]              (PE transpose per row-tile, rhs=diag(q))
    y0n[j,f]   = s[j] x[j,f]
    w1T[f,i]   = sum_j y0n[j,f] ats2[j,i]  = s[i](Mx)[i,f]          (pass 1)
    y1T = y0T - w1T;  y1n = y0n - nat(w1T)      (plain subtracts - q is folded)
    w2T[f,i]   = sum_j y1n[j,f] ats2[j,i]  = s[i](M Tx1)[i,f]       (pass 2)
    y2T = (2 y1T - y0T) - 2 w2T
    outT[fo,i] = sum_k sum_f W[k][f,fo] ykT[f,i]   (lhsT = W natural, rhs = ykT)
    out[i,fo]  = relu(dsq[i] * nat(outT) + bsum)   (dsq = d^0.5)

Performance structure:
  - adj streams via gpsimd SWDGE with fp32->bf16 cast during DMA (8 x 2MB chunks);
    x/consts ride the scalar HWDGE ring (bf16 W/ident prepared host-side);
    out stores ride sync.
  - the per-tile rowsum is split across engines: DVE tensor_reduce on one half,
    ACT activation+accum_out on the other, then dsq = Sqrt(d_a*1 + d_b) via the
    activation bias AP - halves the per-engine cost and latency of the chain
    that gates the diag(q) transposes.
  - pass-1 runs triangularly inside the stream loop; pass-2 per 512-column
    group with a pipelined epilogue (copy, y2T, output matmuls, transpose-back,
    scale+relu, store).
  - PSUM->SBUF copies are [128,1024] ops alternating ACT/DVE (ACT-biased 5:3).
"""

import numpy as np

B, N, F, K = 8, 2048, 128, 3
P = 128
NT = N // P   # 16 node tiles
NCH = NT // 2  # 8 dma chunks of 2 tiles
NCORES = 8

_cache = {}


def _build_nc():
    from contextlib import ExitStack

    import concourse.bacc as bacc
    import concourse.tile as tile
    from concourse import mybir
    from concourse.tile import add_dep_helper

    f32 = mybir.dt.float32
    bf16 = mybir.dt.bfloat16
    AF = mybir.ActivationFunctionType
    OP = mybir.AluOpType
    AX = mybir.AxisListType

    nc = bacc.Bacc("TRN2", target_bir_lowering=False, debug=False, num_devices=NCORES)
    adj = nc.dram_tensor("adj", [N, N], f32, kind="ExternalInput").ap()
    x = nc.dram_tensor("x", [N, F], f32, kind="ExternalInput").ap()
    W = nc.dram_tensor("W", [P, K, F], bf16, kind="ExternalInput").ap()
    bsum_d = nc.dram_tensor("bsum", [P, F], f32, kind="ExternalInput").ap()
    ident = nc.dram_tensor("ident", [P, P], bf16, kind="ExternalInput").ap()
    out = nc.dram_tensor("out", [N, F], f32, kind="ExternalOutput").ap()
    out_t = out.rearrange("(t p) f -> p t f", p=P)
    adj_r = adj.rearrange("(r p) n -> p r n", p=P)
    x_t = x.rearrange("(t p) f -> p t f", p=P)

    with ExitStack() as ctx:
        tc = ctx.enter_context(tile.TileContext(nc))
        consts = ctx.enter_context(tc.tile_pool(name="consts", bufs=1))
        ap_ = ctx.enter_context(tc.tile_pool(name="achunk", bufs=5))
        big = ctx.enter_context(tc.tile_pool(name="big", bufs=1))
        small = ctx.enter_context(tc.tile_pool(name="small", bufs=4))
        scr = ctx.enter_context(tc.tile_pool(name="scr", bufs=2))
        ps_acc = ctx.enter_context(tc.tile_pool(name="ps_acc", bufs=1, space="PSUM"))
        ps_t = ctx.enter_context(tc.tile_pool(name="ps_t", bufs=2, space="PSUM"))

        # ---- constants: gpsimd ring ahead of the adj stream so their
        # completion sems fire early (HWDGE consts starve under the SWDGE
        # flood); x streams in 4 pieces interleaved with the adj chunks;
        # bsum (tail-only) rides the scalar ring.
        ident_bf = consts.tile([P, P], bf16)
        nc.gpsimd.dma_start(out=ident_bf, in_=ident)
        w_bf = consts.tile([P, K, F], bf16)
        nc.gpsimd.dma_start(out=w_bf, in_=W)
        x_f = consts.tile([P, NT, F], f32)
        nc.gpsimd.dma_start(out=x_f[:, 0:4, :], in_=x_t[:, 0:4, :])
        bsum = consts.tile([P, F], f32)
        nc.scalar.dma_start(out=bsum, in_=bsum_d)

        # per-node scalars, [P, NT]: column r holds values for node tile r
        dsq = consts.tile([P, NT], f32)
        sinv = consts.tile([P, NT], f32)

        y0n = big.tile([P, NT, F], bf16)
        y1n = big.tile([P, NT, F], bf16)
        ats2 = big.tile([P, NT, N], bf16)  # [j_in_tile, c(j tile), i]: A[i,j]q[i]
        y0T = big.tile([P, N], bf16)
        y1T = big.tile([P, N], bf16)
        ttT = big.tile([P, N], bf16)
        y2T = big.tile([P, N], bf16)
        w1bf = big.tile([P, N], bf16)
        w2bf = big.tile([P, N], bf16)
        oTbf = big.tile([P, N], bf16)

        z1 = ps_acc.tile([P, N], f32, tag="acc")

        cp = {"i": 0}

        def alt_copy(out, in_):
            # PSUM->SBUF copies: ACT-biased alternation (ACT is cheaper per
            # copy and DVE carries the other per-tile elementwise work)
            cp["i"] += 1
            if cp["i"] % 8 in (0, 2, 4, 5, 6):
                nc.scalar.copy(out=out, in_=in_)
            else:
                nc.vector.tensor_copy(out=out, in_=in_)

        # dummy sqrt so the ACT table load happens in the preamble
        warm = small.tile([P, 1], f32, tag="da")
        nc.vector.memset(warm, 1.0)
        nc.scalar.activation(out=warm, in_=warm, func=AF.Sqrt)

        prev_recip = None

        # ---- streaming phase -------------------------------------------
        # chunk plan: tiles 0 and 1 load alone (compute starts sooner),
        # then 2-tile (2MB) chunks for DMA efficiency
        plan = [(0, 1), (1, 1)] + [(t, 2) for t in range(2, NT, 2)]
        for base, ntile in plan:
            if base in (2, 6, 10):
                # next 256KB x piece, ahead of the chunk whose tiles need it
                k4 = {2: 1, 6: 2, 10: 3}[base]
                nc.gpsimd.dma_start(out=x_f[:, 4 * k4:4 * k4 + 4, :],
                                    in_=x_t[:, 4 * k4:4 * k4 + 4, :])
            a_t = ap_.tile([P, ntile, N], bf16, tag="a")
            nc.gpsimd.dma_start(out=a_t, in_=adj_r[:, base:base + ntile, :])
            for t2 in range(ntile):
                r = base + t2
                at_r = a_t[:, t2, :]
                # rowsum on DVE: fold the halves (bf16 2x mode), then reduce
                half = scr.tile([P, N // 2], bf16, tag="junk")
                i_fold = nc.vector.tensor_tensor(out=half, in0=at_r[:, 0:N // 2],
                                                 in1=at_r[:, N // 2:N], op=OP.add)
                d_a = small.tile([P, 1], f32, tag="da")
                nc.vector.tensor_reduce(out=d_a, in_=half, axis=AX.X, op=OP.add)
                nc.scalar.activation(out=dsq[:, r:r + 1], in_=d_a, func=AF.Sqrt)
                i_recip = nc.vector.reciprocal(out=sinv[:, r:r + 1],
                                               in_=dsq[:, r:r + 1])
                diag2 = small.tile([P, P], bf16, tag="diag")
                nc.vector.tensor_scalar(out=diag2, in0=ident_bf,
                                        scalar1=sinv[:, r:r + 1],
                                        scalar2=sinv[:, r:r + 1],
                                        op0=OP.mult, op1=OP.mult)
                # pin: keep the DVE chain from being queued behind the next
                # tile's DMA-gated fold (shallow pin leaves slack)
                if prev_recip is not None:
                    add_dep_helper(i_fold.ins, prev_recip.ins, sync=False,
                                   reason="keep DVE chain order across tiles")
                prev_recip = i_recip
                # y0n = s * x on ACT (fp32 source, bf16 out, cast folded in)
                nc.scalar.activation(out=y0n[:, r, :], in_=x_f[:, r, :],
                                     func=AF.Identity, scale=sinv[:, r:r + 1])
                # transpose + q[i]-scale A tile row r: 16 (128x128) matmuls
                for g in range(2):
                    pt = ps_t.tile([P, 8, P], f32, tag="t")
                    for qq in range(8):
                        c = 8 * g + qq
                        nc.tensor.matmul(pt[:, qq, :],
                                         lhsT=at_r[:, c * P:(c + 1) * P],
                                         rhs=diag2, start=True, stop=True)
                    nc.scalar.copy(
                        out=ats2[:, 8 * g:8 * g + 8, r * P:(r + 1) * P], in_=pt)

                # transpose y0n tiles into y0T once 4 are ready
                if r % 4 == 3:
                    pt_y0 = ps_t.tile([P, 8, P], f32, tag="t")
                    for qq in range(4):
                        nc.tensor.matmul(pt_y0[:, qq, :], lhsT=y0n[:, r - 3 + qq, :],
                                         rhs=ident_bf, start=True, stop=True)
                    alt_copy(y0T[:, (r - 3) * P:(r + 1) * P], pt_y0[:, 0:4, :])

                # triangular pass-1 terms that became ready with tile r:
                # (a) column block r, strips c <= r.
                # start=True clears has_written for the WHOLE bank (4 column
                # blocks), so only the bank's first-ever matmul may set it.
                for c in range(r + 1):
                    nc.tensor.matmul(z1[:, r * P:(r + 1) * P], lhsT=y0n[:, c, :],
                                     rhs=ats2[:, c, r * P:(r + 1) * P],
                                     start=(r % 4 == 0 and c == 0),
                                     stop=(r == NT - 1 and c == NT - 1),
                                     skip_group_check=True)
                # (b) new strip r into older column blocks (bank chunks)
                for sg in range((r + 3) // 4):
                    lo = 4 * sg
                    hi = min(lo + 4, r)  # blocks [lo, hi)
                    nc.tensor.matmul(z1[:, lo * P:hi * P], lhsT=y0n[:, r, :],
                                     rhs=ats2[:, r, lo * P:hi * P],
                                     start=False, stop=(r == NT - 1),
                                     skip_group_check=True)

        # ---- y1 from w1 = z1: y1T = y0T - w1T; y1n = y0n - nat(w1T) ----
        for g in range(4):
            alt_copy(w1bf[:, g * 512:(g + 1) * 512], z1[:, g * 512:(g + 1) * 512])
        nc.vector.tensor_tensor(out=y1T, in0=y0T, in1=w1bf, op=OP.subtract)
        for g in range(4):
            pt = ps_t.tile([P, 8, P], f32, tag="t")
            for qq in range(4):
                rr = 4 * g + qq
                nc.tensor.matmul(pt[:, qq, :], lhsT=w1bf[:, rr * P:(rr + 1) * P],
                                 rhs=ident_bf, start=True, stop=True)
            nc.vector.tensor_tensor(out=y1n[:, 4 * g:4 * g + 4, :],
                                    in0=y0n[:, 4 * g:4 * g + 4, :],
                                    in1=pt[:, 0:4, :], op=OP.subtract)
        nc.vector.scalar_tensor_tensor(out=ttT, in0=y1T, scalar=2.0, in1=y0T,
                                       op0=OP.mult, op1=OP.subtract)

        # ---- pass 2 by column group with pipelined epilogue ------------
        z2 = ps_acc.tile([P, N], f32, tag="acc")
        ykT = (y0T, y1T, y2T)
        for g in range(4):
            gl, gh = g * 512, (g + 1) * 512
            for c in range(NT):
                nc.tensor.matmul(z2[:, gl:gh], lhsT=y1n[:, c, :],
                                 rhs=ats2[:, c, gl:gh],
                                 start=(c == 0), stop=(c == NT - 1))
            alt_copy(w2bf[:, gl:gh], z2[:, gl:gh])
            nc.vector.scalar_tensor_tensor(out=y2T[:, gl:gh], in0=w2bf[:, gl:gh],
                                           scalar=-2.0, in1=ttT[:, gl:gh],
                                           op0=OP.mult, op1=OP.add)
            oT = ps_t.tile([P, 512], f32, tag="t")
            for k3 in range(K):
                nc.tensor.matmul(oT, lhsT=w_bf[:, k3, :], rhs=ykT[k3][:, gl:gh],
                                 start=(k3 == 0), stop=(k3 == K - 1))
            alt_copy(oTbf[:, gl:gh], oT)
            on = ps_t.tile([P, 8, P], f32, tag="t")
            for qq in range(4):
                rr = 4 * g + qq
                nc.tensor.matmul(on[:, qq, :], lhsT=oTbf[:, rr * P:(rr + 1) * P],
                                 rhs=ident_bf, start=True, stop=True)
            og = small.tile([P, 4, F], f32, tag="og")
            for qq in range(4):
                rr = 4 * g + qq
                tmp = small.tile([P, F], f32, tag="tmp")
                nc.vector.scalar_tensor_tensor(out=tmp, in0=on[:, qq, :],
                                               scalar=dsq[:, rr:rr + 1], in1=bsum,
                                               op0=OP.mult, op1=OP.add)
                nc.scalar.activation(out=og[:, qq, :], in_=tmp, func=AF.Relu)
            nc.sync.dma_start(out=out_t[:, 4 * g:4 * g + 4, :], in_=og)

    nc.compile()
    return nc


def _get_nc():
    if "nc" not in _cache:
        _cache["nc"] = _build_nc()
    return _cache["nc"]


def make_in_maps(x, adj, W, b):
    import ml_dtypes

    bf16 = ml_dtypes.bfloat16
    ident = np.ascontiguousarray(np.eye(P, dtype=np.float32).astype(bf16))
    x = np.ascontiguousarray(np.asarray(x, dtype=np.float32))
    adj = np.ascontiguousarray(np.asarray(adj, dtype=np.float32))
    # [K, in, out] -> [in, K, out], bf16 (the lhsT layout the kernel wants)
    Wf = np.ascontiguousarray(
        np.asarray(W, dtype=np.float32).transpose(1, 0, 2).astype(bf16))
    bf = np.asarray(b, dtype=np.float32)
    bsum = np.ascontiguousarray(
        np.broadcast_to(bf.sum(axis=0), (P, F)).astype(np.float32))
    return [
        {"adj": adj[c], "x": x[c], "W": Wf, "bsum": bsum, "ident": ident}
        for c in range(NCORES)
    ]


def run_raw(x, adj, W, b, **kwargs):
    from concourse import bass_utils

    nc = _get_nc()
    in_maps = make_in_maps(x, adj, W, b)
    res = bass_utils.run_bass_kernel_spmd(nc, in_maps,
                                          core_ids=list(range(NCORES)), **kwargs)
    out = np.stack([res.results[c]["out"] for c in range(NCORES)], axis=0)
    return out.astype(np.float32), res


def kernel(x, adj, W, b):
    out, _ = run_raw(x, adj, W, b)
    return out


# revision 16
# speedup vs baseline: 1.0855x; 1.0850x over previous
"""ChebConv (K=3) kernel for Trainium2, data-parallel over batch across 8 NeuronCores.

Math (per batch b):
    d    = adj.sum(axis=1)  (row sums);  s = d^-0.5;  q = s^2
    M    = Diag(s) A Diag(s);  L = I - M
    Tx0 = x, Tx1 = x - Mx, Tx2 = 2(Tx1 - M Tx1) - x
    out  = relu(sum_k Txk @ W[k] + bsum)
  (the reference's +1e-6 under the sqrt is ~1e-9 relative for this input
   distribution - far below bf16 noise - so it is folded away here.)

Kernel-side dataflow (everything bf16 except PSUM accum; yk := Diag(s) Txk):
    ats2[j,i]  = A[i,j] q[i]              (PE transpose per row-tile, rhs=diag(q))
    y0n[j,f]   = s[j] x[j,f]
    w1T[f,i]   = sum_j y0n[j,f] ats2[j,i]  = s[i](Mx)[i,f]          (pass 1)
    y1T = y0T - w1T;  y1n = y0n - nat(w1T)      (plain subtracts - q is folded)
    w2T[f,i]   = sum_j y1n[j,f] ats2[j,i]  = s[i](M Tx1)[i,f]       (pass 2)
    y2T = (2 y1T - y0T) - 2 w2T
    outT[fo,i] = sum_k sum_f W[k][f,fo] ykT[f,i]   (lhsT = W natural, rhs = ykT)
    out[i,fo]  = relu(dsq[i] * nat(outT) + bsum)   (dsq = d^0.5)

Performance structure:
  - adj streams via gpsimd SWDGE with fp32->bf16 cast during DMA (8 x 2MB chunks);
    x/consts ride the scalar HWDGE ring (bf16 W/ident prepared host-side);
    out stores ride sync.
  - the per-tile rowsum is split across engines: DVE tensor_reduce on one half,
    ACT activation+accum_out on the other, then dsq = Sqrt(d_a*1 + d_b) via the
    activation bias AP - halves the per-engine cost and latency of the chain
    that gates the diag(q) transposes.
  - pass-1 runs triangularly inside the stream loop; pass-2 per 512-column
    group with a pipelined epilogue (copy, y2T, output matmuls, transpose-back,
    scale+relu, store).
  - PSUM->SBUF copies are [128,1024] ops alternating ACT/DVE (ACT-biased 5:3).
"""

import numpy as np

B, N, F, K = 8, 2048, 128, 3
P = 128
NT = N // P   # 16 node tiles
NCH = NT // 2  # 8 dma chunks of 2 tiles
NCORES = 8

_cache = {}


def _build_nc():
    from contextlib import ExitStack

    import concourse.bacc as bacc
    import concourse.tile as tile
    from concourse import mybir
    from concourse.tile import add_dep_helper

    f32 = mybir.dt.float32
    bf16 = mybir.dt.bfloat16
    AF = mybir.ActivationFunctionType
    OP = mybir.AluOpType
    AX = mybir.AxisListType

    nc = bacc.Bacc("TRN2", target_bir_lowering=False, debug=False, num_devices=NCORES)
    adj = nc.dram_tensor("adj", [N, N], f32, kind="ExternalInput").ap()
    x = nc.dram_tensor("x", [N, F], f32, kind="ExternalInput").ap()
    W = nc.dram_tensor("W", [P, K, F], bf16, kind="ExternalInput").ap()
    bsum_d = nc.dram_tensor("bsum", [P, F], f32, kind="ExternalInput").ap()
    ident = nc.dram_tensor("ident", [P, P], bf16, kind="ExternalInput").ap()
    out = nc.dram_tensor("out", [N, F], f32, kind="ExternalOutput").ap()
    out_t = out.rearrange("(t p) f -> p t f", p=P)
    adj_r = adj.rearrange("(r p) n -> p r n", p=P)
    x_t = x.rearrange("(t p) f -> p t f", p=P)

    with ExitStack() as ctx:
        tc = ctx.enter_context(tile.TileContext(nc))
        consts = ctx.enter_context(tc.tile_pool(name="consts", bufs=1))
        ap_ = ctx.enter_context(tc.tile_pool(name="achunk", bufs=5))
        big = ctx.enter_context(tc.tile_pool(name="big", bufs=1))
        small = ctx.enter_context(tc.tile_pool(name="small", bufs=4))
        scr = ctx.enter_context(tc.tile_pool(name="scr", bufs=2))
        ps_acc = ctx.enter_context(tc.tile_pool(name="ps_acc", bufs=1, space="PSUM"))
        ps_t = ctx.enter_context(tc.tile_pool(name="ps_t", bufs=2, space="PSUM"))

        # ---- constants: gpsimd ring ahead of the adj stream so their
        # completion sems fire early (HWDGE consts starve under the SWDGE
        # flood); x streams in 4 pieces interleaved with the adj chunks;
        # bsum (tail-only) rides the scalar ring.
        ident_bf = consts.tile([P, P], bf16)
        nc.gpsimd.dma_start(out=ident_bf, in_=ident)
        w_bf = consts.tile([P, K, F], bf16)
        nc.gpsimd.dma_start(out=w_bf, in_=W)
        x_f = consts.tile([P, NT, F], f32)
        nc.gpsimd.dma_start(out=x_f[:, 0:4, :], in_=x_t[:, 0:4, :])
        bsum = consts.tile([P, F], f32)
        nc.scalar.dma_start(out=bsum, in_=bsum_d)

        # per-node scalars, [P, NT]: column r holds values for node tile r
        dsq = consts.tile([P, NT], f32)
        sinv = consts.tile([P, NT], f32)

        y0n = big.tile([P, NT, F], bf16)
        y1n = big.tile([P, NT, F], bf16)
        ats2 = big.tile([P, NT, N], bf16)  # [j_in_tile, c(j tile), i]: A[i,j]q[i]
        y0T = big.tile([P, N], bf16)
        y1T = big.tile([P, N], bf16)
        ttT = big.tile([P, N], bf16)
        y2T = big.tile([P, N], bf16)
        w1bf = big.tile([P, N], bf16)
        w2bf = big.tile([P, N], bf16)
        oTbf = big.tile([P, N], bf16)

        z1 = ps_acc.tile([P, N], f32, tag="acc")

        cp = {"i": 0}

        def alt_copy(out, in_):
            # PSUM->SBUF copies: ACT-biased alternation (ACT is cheaper per
            # copy and DVE carries the other per-tile elementwise work)
            cp["i"] += 1
            if cp["i"] % 8 in (0, 2, 4, 5, 6):
                nc.scalar.copy(out=out, in_=in_)
            else:
                nc.vector.tensor_copy(out=out, in_=in_)

        # dummy sqrt so the ACT table load happens in the preamble
        warm = small.tile([P, 1], f32, tag="da")
        nc.vector.memset(warm, 1.0)
        nc.scalar.activation(out=warm, in_=warm, func=AF.Sqrt)

        recips = []

        # ---- streaming phase -------------------------------------------
        # chunk plan: tiles 0 and 1 load alone (compute starts sooner),
        # then 2-tile (2MB) chunks for DMA efficiency
        plan = [(0, 1), (1, 1)] + [(t, 2) for t in range(2, NT, 2)]
        for base, ntile in plan:
            if base in (2, 6, 10):
                # next 256KB x piece, ahead of the chunk whose tiles need it
                k4 = {2: 1, 6: 2, 10: 3}[base]
                nc.gpsimd.dma_start(out=x_f[:, 4 * k4:4 * k4 + 4, :],
                                    in_=x_t[:, 4 * k4:4 * k4 + 4, :])
            a_t = ap_.tile([P, ntile, N], bf16, tag="a")
            nc.gpsimd.dma_start(out=a_t, in_=adj_r[:, base:base + ntile, :])
            for t2 in range(ntile):
                r = base + t2
                at_r = a_t[:, t2, :]
                # rowsum on DVE: fold the halves (bf16 2x mode), then reduce
                half = scr.tile([P, N // 2], bf16, tag="junk")
                i_fold = nc.vector.tensor_tensor(out=half, in0=at_r[:, 0:N // 2],
                                                 in1=at_r[:, N // 2:N], op=OP.add)
                d_a = small.tile([P, 1], f32, tag="da")
                nc.vector.tensor_reduce(out=d_a, in_=half, axis=AX.X, op=OP.add)
                nc.scalar.activation(out=dsq[:, r:r + 1], in_=d_a, func=AF.Sqrt)
                i_recip = nc.vector.reciprocal(out=sinv[:, r:r + 1],
                                               in_=dsq[:, r:r + 1])
                diag2 = small.tile([P, P], bf16, tag="diag")
                nc.vector.tensor_scalar(out=diag2, in0=ident_bf,
                                        scalar1=sinv[:, r:r + 1],
                                        scalar2=sinv[:, r:r + 1],
                                        op0=OP.mult, op1=OP.mult)
                # pin with two tiles of slack: tile r's fold may not jump
                # ahead of tile r-2's reciprocal in the DVE queue (prevents
                # DMA-gated folds from blocking older scalar chains) while
                # still letting two chains overlap across engines
                if len(recips) >= 2:
                    add_dep_helper(i_fold.ins, recips[-2].ins, sync=False,
                                   reason="keep DVE chain order across tiles")
                recips.append(i_recip)
                # y0n = s * x on ACT (fp32 source, bf16 out, cast folded in)
                nc.scalar.activation(out=y0n[:, r, :], in_=x_f[:, r, :],
                                     func=AF.Identity, scale=sinv[:, r:r + 1])
                # transpose + q[i]-scale A tile row r: 16 (128x128) matmuls
                for g in range(2):
                    pt = ps_t.tile([P, 8, P], f32, tag="t")
                    for qq in range(8):
                        c = 8 * g + qq
                        nc.tensor.matmul(pt[:, qq, :],
                                         lhsT=at_r[:, c * P:(c + 1) * P],
                                         rhs=diag2, start=True, stop=True)
                    nc.scalar.copy(
                        out=ats2[:, 8 * g:8 * g + 8, r * P:(r + 1) * P], in_=pt)

                # transpose y0n tiles into y0T once 4 are ready
                if r % 4 == 3:
                    pt_y0 = ps_t.tile([P, 8, P], f32, tag="t")
                    for qq in range(4):
                        nc.tensor.matmul(pt_y0[:, qq, :], lhsT=y0n[:, r - 3 + qq, :],
                                         rhs=ident_bf, start=True, stop=True)
                    alt_copy(y0T[:, (r - 3) * P:(r + 1) * P], pt_y0[:, 0:4, :])

                # triangular pass-1 terms that became ready with tile r:
                # (a) column block r, strips c <= r.
                # start=True clears has_written for the WHOLE bank (4 column
                # blocks), so only the bank's first-ever matmul may set it.
                for c in range(r + 1):
                    nc.tensor.matmul(z1[:, r * P:(r + 1) * P], lhsT=y0n[:, c, :],
                                     rhs=ats2[:, c, r * P:(r + 1) * P],
                                     start=(r % 4 == 0 and c == 0),
                                     stop=(r == NT - 1 and c == NT - 1),
                                     skip_group_check=True)
                # (b) new strip r into older column blocks (bank chunks)
                for sg in range((r + 3) // 4):
                    lo = 4 * sg
                    hi = min(lo + 4, r)  # blocks [lo, hi)
                    nc.tensor.matmul(z1[:, lo * P:hi * P], lhsT=y0n[:, r, :],
                                     rhs=ats2[:, r, lo * P:hi * P],
                                     start=False, stop=(r == NT - 1),
                                     skip_group_check=True)

        # ---- y1 from w1 = z1: y1T = y0T - w1T; y1n = y0n - nat(w1T) ----
        for g in range(4):
            alt_copy(w1bf[:, g * 512:(g + 1) * 512], z1[:, g * 512:(g + 1) * 512])
        nc.vector.tensor_tensor(out=y1T, in0=y0T, in1=w1bf, op=OP.subtract)
        for g in range(4):
            pt = ps_t.tile([P, 8, P], f32, tag="t")
            for qq in range(4):
                rr = 4 * g + qq
                nc.tensor.matmul(pt[:, qq, :], lhsT=w1bf[:, rr * P:(rr + 1) * P],
                                 rhs=ident_bf, start=True, stop=True)
            nc.vector.tensor_tensor(out=y1n[:, 4 * g:4 * g + 4, :],
                                    in0=y0n[:, 4 * g:4 * g + 4, :],
                                    in1=pt[:, 0:4, :], op=OP.subtract)
        nc.vector.scalar_tensor_tensor(out=ttT, in0=y1T, scalar=2.0, in1=y0T,
                                       op0=OP.mult, op1=OP.subtract)

        # ---- pass 2 by column group with pipelined epilogue ------------
        z2 = ps_acc.tile([P, N], f32, tag="acc")
        ykT = (y0T, y1T, y2T)
        for g in range(4):
            gl, gh = g * 512, (g + 1) * 512
            for c in range(NT):
                nc.tensor.matmul(z2[:, gl:gh], lhsT=y1n[:, c, :],
                                 rhs=ats2[:, c, gl:gh],
                                 start=(c == 0), stop=(c == NT - 1))
            alt_copy(w2bf[:, gl:gh], z2[:, gl:gh])
            nc.vector.scalar_tensor_tensor(out=y2T[:, gl:gh], in0=w2bf[:, gl:gh],
                                           scalar=-2.0, in1=ttT[:, gl:gh],
                                           op0=OP.mult, op1=OP.add)
            oT = ps_t.tile([P, 512], f32, tag="t")
            for k3 in range(K):
                nc.tensor.matmul(oT, lhsT=w_bf[:, k3, :], rhs=ykT[k3][:, gl:gh],
                                 start=(k3 == 0), stop=(k3 == K - 1))
            alt_copy(oTbf[:, gl:gh], oT)
            on = ps_t.tile([P, 8, P], f32, tag="t")
            for qq in range(4):
                rr = 4 * g + qq
                nc.tensor.matmul(on[:, qq, :], lhsT=oTbf[:, rr * P:(rr + 1) * P],
                                 rhs=ident_bf, start=True, stop=True)
            og = small.tile([P, 4, F], f32, tag="og")
            for qq in range(4):
                rr = 4 * g + qq
                tmp = small.tile([P, F], f32, tag="tmp")
                nc.vector.scalar_tensor_tensor(out=tmp, in0=on[:, qq, :],
                                               scalar=dsq[:, rr:rr + 1], in1=bsum,
                                               op0=OP.mult, op1=OP.add)
                nc.scalar.activation(out=og[:, qq, :], in_=tmp, func=AF.Relu)
            nc.sync.dma_start(out=out_t[:, 4 * g:4 * g + 4, :], in_=og)

    nc.compile()
    return nc


def _get_nc():
    if "nc" not in _cache:
        _cache["nc"] = _build_nc()
    return _cache["nc"]


def make_in_maps(x, adj, W, b):
    import ml_dtypes

    bf16 = ml_dtypes.bfloat16
    ident = np.ascontiguousarray(np.eye(P, dtype=np.float32).astype(bf16))
    x = np.ascontiguousarray(np.asarray(x, dtype=np.float32))
    adj = np.ascontiguousarray(np.asarray(adj, dtype=np.float32))
    # [K, in, out] -> [in, K, out], bf16 (the lhsT layout the kernel wants)
    Wf = np.ascontiguousarray(
        np.asarray(W, dtype=np.float32).transpose(1, 0, 2).astype(bf16))
    bf = np.asarray(b, dtype=np.float32)
    bsum = np.ascontiguousarray(
        np.broadcast_to(bf.sum(axis=0), (P, F)).astype(np.float32))
    return [
        {"adj": adj[c], "x": x[c], "W": Wf, "bsum": bsum, "ident": ident}
        for c in range(NCORES)
    ]


def run_raw(x, adj, W, b, **kwargs):
    from concourse import bass_utils

    nc = _get_nc()
    in_maps = make_in_maps(x, adj, W, b)
    res = bass_utils.run_bass_kernel_spmd(nc, in_maps,
                                          core_ids=list(range(NCORES)), **kwargs)
    out = np.stack([res.results[c]["out"] for c in range(NCORES)], axis=0)
    return out.astype(np.float32), res


def kernel(x, adj, W, b):
    out, _ = run_raw(x, adj, W, b)
    return out
